# revision 2
# baseline (speedup 1.0000x reference)
"""LIF_R single-step kernel for Trainium2, 8-core SPMD.

Strategy: row-shard w (8192x8192 f32) across 8 cores (1024 rows each).
Each core streams its 32MB w-shard through SBUF in [128 x 2048] tiles
(contiguous per-partition DMA reads) and computes the matvec row-sums
with fused DVE tensor_tensor_reduce against a partition-broadcast copy
of g. The elementwise LIF dynamics run on [128, 8] per-partition
layouts; host reassembles the 8 x 1024 output slices.
"""
import sys

if "/opt/trn_rl_repo" not in sys.path:
    sys.path.insert(0, "/opt/trn_rl_repo")

import numpy as np

N = 8192
N_CORES = 8
ROWS = N // N_CORES          # 1024 rows per core
M_TILES = ROWS // 128        # 8 partition tiles per core
K_C = 2048                   # k-chunk per DMA/TTR (1MB tiles)
N_K = N // K_C               # 4 k-chunks per m-tile

R_I = 18.0
F_V = 0.12
DELTA_V = 12.0

_COMPILED = None
LAST_RESULT = None


def _build():
    import concourse.bacc as bacc
    import concourse.tile as tile
    import concourse.mybir as mybir

    f32 = mybir.dt.float32
    Alu = mybir.AluOpType

    nc = bacc.Bacc("TRN2", target_bir_lowering=False, debug=False,
                   num_devices=N_CORES)

    w_d = nc.dram_tensor("w", [ROWS, N], f32, kind="ExternalInput")
    g_d = nc.dram_tensor("g", [1, N], f32, kind="ExternalInput")
    x_d = nc.dram_tensor("x", [128, M_TILES], f32, kind="ExternalInput")
    v_d = nc.dram_tensor("v", [128, M_TILES], f32, kind="ExternalInput")
    th_d = nc.dram_tensor("th", [128, M_TILES], f32, kind="ExternalInput")
    el_d = nc.dram_tensor("el", [128, M_TILES], f32, kind="ExternalInput")
    cm_d = nc.dram_tensor("cm", [128, M_TILES], f32, kind="ExternalInput")
    gg_d = nc.dram_tensor("gg", [128, M_TILES], f32, kind="ExternalInput")
    vout_d = nc.dram_tensor("v_out", [128, M_TILES], f32, kind="ExternalOutput")
    sout_d = nc.dram_tensor("s_out", [128, M_TILES], f32, kind="ExternalOutput")

    with tile.TileContext(nc) as tc:
        with (
            tc.tile_pool(name="gpool", bufs=1) as gpool,
            tc.tile_pool(name="wpool", bufs=4) as wpool,
            tc.tile_pool(name="spool", bufs=2) as spool,
            tc.tile_pool(name="small", bufs=1) as small,
        ):
            g_row = gpool.tile([1, N], f32)
            nc.sync.dma_start(g_row[:], g_d.ap())
            g_b = gpool.tile([128, N], f32)
            nc.gpsimd.partition_broadcast(g_b[:], g_row[0:1, :])

            acc = small.tile([128, M_TILES * N_K], f32)
            w_ap = w_d.ap()
            for m in range(M_TILES):
                for kc in range(N_K):
                    w_t = wpool.tile([128, K_C], f32)
                    nc.sync.dma_start(
                        w_t[:],
                        w_ap[m * 128:(m + 1) * 128, kc * K_C:(kc + 1) * K_C],
                    )
                    scratch = spool.tile([128, K_C], f32)
                    nc.vector.scalar_tensor_tensor(
                        out=scratch[:],
                        in0=w_t[:],
                        scalar=0.0,
                        in1=g_b[:, kc * K_C:(kc + 1) * K_C],
                        op0=Alu.bypass,
                        op1=Alu.mult,
                        accum_out=acc[:, m * N_K + kc: m * N_K + kc + 1],
                    )

            # epilogue on [128, M_TILES] per-partition layouts
            x_t = small.tile([128, M_TILES], f32)
            nc.sync.dma_start(x_t[:], x_d.ap())
            v_t = small.tile([128, M_TILES], f32)
            nc.sync.dma_start(v_t[:], v_d.ap())
            th_t = small.tile([128, M_TILES], f32)
            nc.sync.dma_start(th_t[:], th_d.ap())
            el_t = small.tile([128, M_TILES], f32)
            nc.sync.dma_start(el_t[:], el_d.ap())
            cm_t = small.tile([128, M_TILES], f32)
            nc.sync.dma_start(cm_t[:], cm_d.ap())
            gg_t = small.tile([128, M_TILES], f32)
            nc.sync.dma_start(gg_t[:], gg_d.ap())

            I_t = small.tile([128, M_TILES], f32)
            for m in range(M_TILES):
                nc.vector.tensor_reduce(
                    out=I_t[:, m:m + 1],
                    in_=acc[:, m * N_K:(m + 1) * N_K],
                    axis=mybir.AxisListType.X,
                    op=Alu.add,
                )
            nc.vector.tensor_add(I_t[:], I_t[:], x_t[:])

            t1 = small.tile([128, M_TILES], f32)
            nc.vector.tensor_sub(t1[:], el_t[:], v_t[:])
            nc.vector.tensor_mul(t1[:], t1[:], gg_t[:])
            num = small.tile([128, M_TILES], f32)
            nc.vector.scalar_tensor_tensor(
                out=num[:], in0=I_t[:], scalar=R_I, in1=t1[:],
                op0=Alu.mult, op1=Alu.add,
            )
            rc = small.tile([128, M_TILES], f32)
            nc.vector.reciprocal(rc[:], cm_t[:])
            v_next = small.tile([128, M_TILES], f32)
            nc.vector.tensor_mul(v_next[:], num[:], rc[:])
            nc.vector.tensor_add(v_next[:], v_next[:], v_t[:])

            s_arg = small.tile([128, M_TILES], f32)
            nc.vector.tensor_sub(s_arg[:], v_next[:], th_t[:])
            soft = small.tile([128, M_TILES], f32)
            bias0 = small.tile([128, 1], f32)
            nc.gpsimd.memset(bias0[:], 0.0)
            nc.scalar.activation(
                soft[:], s_arg[:], mybir.ActivationFunctionType.Sigmoid,
                bias=bias0[:],
            )

            spk = small.tile([128, M_TILES], f32)
            nc.vector.tensor_tensor(spk[:], v_next[:], th_t[:], Alu.is_ge)

            vr = small.tile([128, M_TILES], f32)
            nc.vector.tensor_sub(vr[:], v_t[:], el_t[:])
            nc.vector.scalar_tensor_tensor(
                out=vr[:], in0=vr[:], scalar=F_V, in1=el_t[:],
                op0=Alu.mult, op1=Alu.add,
            )
            nc.vector.tensor_scalar(
                out=vr[:], in0=vr[:], scalar1=-DELTA_V, scalar2=None,
                op0=Alu.add,
            )
            nc.vector.tensor_sub(vr[:], vr[:], v_next[:])
            nc.vector.tensor_mul(vr[:], vr[:], spk[:])
            v_new = small.tile([128, M_TILES], f32)
            nc.vector.tensor_add(v_new[:], v_next[:], vr[:])

            nc.sync.dma_start(vout_d.ap(), v_new[:])
            nc.sync.dma_start(sout_d.ap(), soft[:])

    nc.compile()
    return nc


def kernel(x_in, v, g, theta_s, w, E_L, C_m, G, tau_g):
    global _COMPILED, LAST_RESULT
    from concourse import bass_utils

    if _COMPILED is None:
        _COMPILED = _build()
    nc = _COMPILED

    x_in = np.asarray(x_in, dtype=np.float32)
    v = np.asarray(v, dtype=np.float32)
    g = np.asarray(g, dtype=np.float32)
    theta_s = np.asarray(theta_s, dtype=np.float32)
    w = np.asarray(w, dtype=np.float32)
    E_L = np.asarray(E_L, dtype=np.float32)
    C_m = np.asarray(C_m, dtype=np.float32)
    G = np.asarray(G, dtype=np.float32)

    g2 = np.ascontiguousarray(g.reshape(1, N))
    in_maps = []
    for c in range(N_CORES):
        sl = slice(ROWS * c, ROWS * (c + 1))

        def col(a):
            return np.ascontiguousarray(a[sl].reshape(M_TILES, 128).T)

        in_maps.append({
            "w": np.ascontiguousarray(w[sl]),
            "g": g2,
            "x": col(x_in),
            "v": col(v),
            "th": col(theta_s),
            "el": col(E_L),
            "cm": col(C_m),
            "gg": col(G),
        })

    res = bass_utils.run_bass_kernel_spmd(
        nc, in_maps, core_ids=list(range(N_CORES)))
    LAST_RESULT = res
    v_new = np.concatenate(
        [res.results[c]["v_out"].T.ravel() for c in range(N_CORES)])
    soft = np.concatenate(
        [res.results[c]["s_out"].T.ravel() for c in range(N_CORES)])
    return v_new.astype(np.float32), soft.astype(np.float32)


# revision 4
# speedup vs baseline: 1.2886x; 1.2886x over previous
"""LIF_R single-step kernel for Trainium2, 8-core SPMD.

Strategy: row-shard w (8192x8192 f32) across 8 cores (1024 rows each).
Each core streams its 32MB w-shard through SBUF in [128 x 2048] tiles
(contiguous per-partition DMA reads) and computes the matvec row-sums
with fused DVE tensor_tensor_reduce against a partition-broadcast copy
of g. The elementwise LIF dynamics run on [128, 8] per-partition
layouts; host reassembles the 8 x 1024 output slices.
"""
import sys

if "/opt/trn_rl_repo" not in sys.path:
    sys.path.insert(0, "/opt/trn_rl_repo")

import numpy as np

N = 8192
N_CORES = 8
ROWS = N // N_CORES          # 1024 rows per core
M_TILES = ROWS // 128        # 8 partition tiles per core
K_C = 2048                   # k-chunk per DMA/TTR (1MB tiles)
N_K = N // K_C               # 4 k-chunks per m-tile

R_I = 18.0
F_V = 0.12
DELTA_V = 12.0

_COMPILED = None
LAST_RESULT = None


def _build():
    import concourse.bacc as bacc
    import concourse.tile as tile
    import concourse.mybir as mybir

    f32 = mybir.dt.float32
    Alu = mybir.AluOpType

    nc = bacc.Bacc("TRN2", target_bir_lowering=False, debug=False,
                   num_devices=N_CORES)

    w_d = nc.dram_tensor("w", [ROWS, N], f32, kind="ExternalInput")
    g_d = nc.dram_tensor("g", [1, N], f32, kind="ExternalInput")
    x_d = nc.dram_tensor("x", [128, M_TILES], f32, kind="ExternalInput")
    v_d = nc.dram_tensor("v", [128, M_TILES], f32, kind="ExternalInput")
    th_d = nc.dram_tensor("th", [128, M_TILES], f32, kind="ExternalInput")
    el_d = nc.dram_tensor("el", [128, M_TILES], f32, kind="ExternalInput")
    cm_d = nc.dram_tensor("cm", [128, M_TILES], f32, kind="ExternalInput")
    gg_d = nc.dram_tensor("gg", [128, M_TILES], f32, kind="ExternalInput")
    vout_d = nc.dram_tensor("v_out", [128, M_TILES], f32, kind="ExternalOutput")
    sout_d = nc.dram_tensor("s_out", [128, M_TILES], f32, kind="ExternalOutput")

    with tile.TileContext(nc) as tc:
        with (
            tc.tile_pool(name="gpool", bufs=1) as gpool,
            tc.tile_pool(name="wpool", bufs=6) as wpool,
            tc.tile_pool(name="spool", bufs=2) as spool,
            tc.tile_pool(name="small", bufs=1) as small,
        ):
            g_row = gpool.tile([1, N], f32)
            nc.sync.dma_start(g_row[:], g_d.ap())
            g_b = []
            for kc in range(N_K):
                gb = gpool.tile([128, K_C], f32, tag=f"gb{kc}")
                nc.gpsimd.partition_broadcast(
                    gb[:], g_row[0:1, kc * K_C:(kc + 1) * K_C])
                g_b.append(gb)

            acc = small.tile([128, M_TILES * N_K], f32)
            w_ap = w_d.ap()
            for kc in range(N_K):
                for m in range(M_TILES):
                    w_t = wpool.tile([128, K_C], f32)
                    nc.sync.dma_start(
                        w_t[:],
                        w_ap[m * 128:(m + 1) * 128, kc * K_C:(kc + 1) * K_C],
                    )
                    scratch = spool.tile([128, K_C], f32)
                    nc.vector.scalar_tensor_tensor(
                        out=scratch[:],
                        in0=w_t[:],
                        scalar=0.0,
                        in1=g_b[kc][:],
                        op0=Alu.bypass,
                        op1=Alu.mult,
                        accum_out=acc[:, m * N_K + kc: m * N_K + kc + 1],
                    )

            # epilogue on [128, M_TILES] per-partition layouts
            x_t = small.tile([128, M_TILES], f32)
            nc.sync.dma_start(x_t[:], x_d.ap())
            v_t = small.tile([128, M_TILES], f32)
            nc.sync.dma_start(v_t[:], v_d.ap())
            th_t = small.tile([128, M_TILES], f32)
            nc.sync.dma_start(th_t[:], th_d.ap())
            el_t = small.tile([128, M_TILES], f32)
            nc.sync.dma_start(el_t[:], el_d.ap())
            cm_t = small.tile([128, M_TILES], f32)
            nc.sync.dma_start(cm_t[:], cm_d.ap())
            gg_t = small.tile([128, M_TILES], f32)
            nc.sync.dma_start(gg_t[:], gg_d.ap())

            I_t = small.tile([128, M_TILES], f32)
            for m in range(M_TILES):
                nc.vector.tensor_reduce(
                    out=I_t[:, m:m + 1],
                    in_=acc[:, m * N_K:(m + 1) * N_K],
                    axis=mybir.AxisListType.X,
                    op=Alu.add,
                )
            nc.vector.tensor_add(I_t[:], I_t[:], x_t[:])

            t1 = small.tile([128, M_TILES], f32)
            nc.vector.tensor_sub(t1[:], el_t[:], v_t[:])
            nc.vector.tensor_mul(t1[:], t1[:], gg_t[:])
            num = small.tile([128, M_TILES], f32)
            nc.vector.scalar_tensor_tensor(
                out=num[:], in0=I_t[:], scalar=R_I, in1=t1[:],
                op0=Alu.mult, op1=Alu.add,
            )
            rc = small.tile([128, M_TILES], f32)
            nc.vector.reciprocal(rc[:], cm_t[:])
            v_next = small.tile([128, M_TILES], f32)
            nc.vector.tensor_mul(v_next[:], num[:], rc[:])
            nc.vector.tensor_add(v_next[:], v_next[:], v_t[:])

            s_arg = small.tile([128, M_TILES], f32)
            nc.vector.tensor_sub(s_arg[:], v_next[:], th_t[:])
            soft = small.tile([128, M_TILES], f32)
            bias0 = small.tile([128, 1], f32)
            nc.gpsimd.memset(bias0[:], 0.0)
            nc.scalar.activation(
                soft[:], s_arg[:], mybir.ActivationFunctionType.Sigmoid,
                bias=bias0[:],
            )

            spk = small.tile([128, M_TILES], f32)
            nc.vector.tensor_tensor(spk[:], v_next[:], th_t[:], Alu.is_ge)

            vr = small.tile([128, M_TILES], f32)
            nc.vector.tensor_sub(vr[:], v_t[:], el_t[:])
            nc.vector.scalar_tensor_tensor(
                out=vr[:], in0=vr[:], scalar=F_V, in1=el_t[:],
                op0=Alu.mult, op1=Alu.add,
            )
            nc.vector.tensor_scalar(
                out=vr[:], in0=vr[:], scalar1=-DELTA_V, scalar2=None,
                op0=Alu.add,
            )
            nc.vector.tensor_sub(vr[:], vr[:], v_next[:])
            nc.vector.tensor_mul(vr[:], vr[:], spk[:])
            v_new = small.tile([128, M_TILES], f32)
            nc.vector.tensor_add(v_new[:], v_next[:], vr[:])

            nc.sync.dma_start(vout_d.ap(), v_new[:])
            nc.sync.dma_start(sout_d.ap(), soft[:])

    nc.compile()
    return nc


def kernel(x_in, v, g, theta_s, w, E_L, C_m, G, tau_g):
    global _COMPILED, LAST_RESULT
    from concourse import bass_utils

    if _COMPILED is None:
        _COMPILED = _build()
    nc = _COMPILED

    x_in = np.asarray(x_in, dtype=np.float32)
    v = np.asarray(v, dtype=np.float32)
    g = np.asarray(g, dtype=np.float32)
    theta_s = np.asarray(theta_s, dtype=np.float32)
    w = np.asarray(w, dtype=np.float32)
    E_L = np.asarray(E_L, dtype=np.float32)
    C_m = np.asarray(C_m, dtype=np.float32)
    G = np.asarray(G, dtype=np.float32)

    g2 = np.ascontiguousarray(g.reshape(1, N))
    in_maps = []
    for c in range(N_CORES):
        sl = slice(ROWS * c, ROWS * (c + 1))

        def col(a):
            return np.ascontiguousarray(a[sl].reshape(M_TILES, 128).T)

        in_maps.append({
            "w": np.ascontiguousarray(w[sl]),
            "g": g2,
            "x": col(x_in),
            "v": col(v),
            "th": col(theta_s),
            "el": col(E_L),
            "cm": col(C_m),
            "gg": col(G),
        })

    res = bass_utils.run_bass_kernel_spmd(
        nc, in_maps, core_ids=list(range(N_CORES)))
    LAST_RESULT = res
    v_new = np.concatenate(
        [res.results[c]["v_out"].T.ravel() for c in range(N_CORES)])
    soft = np.concatenate(
        [res.results[c]["s_out"].T.ravel() for c in range(N_CORES)])
    return v_new.astype(np.float32), soft.astype(np.float32)
